# revision 12
# baseline (speedup 1.0000x reference)
"""Balanced GIF neuron kernel for TRN2 (8 NeuronCores, data-parallel over batch).

Reference computation (per batch b, time t):
    h = x @ W^T + b ; i = concat(relu(h_exc), -relu(h_inh))
    scan over t: v = clip(v*DECAY + i, +-32*theta); s = clip(floor(v/theta),0,16)
                 v -= s*theta; theta += a*s - a*(theta-1)

Design: core c owns batches [2c, 2c+1].  PE does the (2048 x 1024 x 2048)
projection per core in fp32 (W^T and x^T staged via PE transposes); the
1024-step recurrence runs on the DVE with 10 instructions per step over a
(128, 16, 2) lane tile (partition = h%128, free = (h//128, b)), using three
runtime-registered custom DVE ops (spike clip-floor, post-spike clamp, theta
update) plus the stock reciprocal_approx_accurate pair for 1/theta.

Key algebraic identity used (exactly equivalent to the reference in f32):
    v' = clip(v1 - s*theta, -32*theta, 16*theta)
matches clamp-then-spike-subtract of the reference for every case.
"""

import math
import re

import numpy as np

# ---- model constants (match reference.py) ----
INPUT_DIM = 1024
HIDDEN_DIM = 2048
L = 16.0
DT = 1.0
TAU = 10.0
THRESHOLD = 1.0
ALPHA = 0.01
EXC_DIM = 1638
INH_DIM = 410
DECAY = math.exp(-DT / TAU)

MAGIC = 8388608.0  # 2**23: (q + MAGIC) - MAGIC = round-to-nearest-even integer
B_FULL, T_LEN = 16, 1024
N_CORES = 8
B_PER = B_FULL // N_CORES          # 2 batches per core
NT = 16                            # h tiles of 128
TC = 128                           # time chunk
N_CHUNK = T_LEN // TC
KT = INPUT_DIM // 128              # 8 k tiles
LANES_F = (NT, B_PER)              # free dims of the scan lane tile

_NC_CACHE = {}


def _register_custom_ops():
    from concourse.dve_ops import (
        OPS,
        CUSTOM_DVE_SPECS,
        DveOp,
        _CUSTOM_DVE_ROW_BASE,
        _SUB_OPCODE_FOR_NAME,
    )
    from concourse.dve_spec import C0, C1, One, Spec, Src0, Src1, Zero, maxx, minn

    def reg(name, spec, ref):
        if name in _SUB_OPCODE_FOR_NAME:
            return next(op for op in OPS if op.name == name)
        spec = Spec(body=spec.body, reference=ref)
        row = _CUSTOM_DVE_ROW_BASE + len(OPS)
        assert row < 0x20, "custom DVE op rows exhausted"
        tmp = DveOp(name, spec, False, uops_sha={})
        OPS.append(tmp)
        _SUB_OPCODE_FOR_NAME[name] = row
        CUSTOM_DVE_SPECS[name] = spec
        shas = {}
        for ver in ("v3", "v4"):
            try:
                tmp.compile(ver)
            except ValueError as e:
                m = re.search(r"([0-9a-f]{8,}) ", str(e))
                shas[ver] = m.group(1)
        real = DveOp(name, spec, False, uops_sha=shas)
        OPS[-1] = real
        CUSTOM_DVE_SPECS[name] = spec
        return real

    # s = clip(rf - (rf > q), 0, s0)   (rf=Src0 = rne-round(q), q=Src1)
    spike = reg(
        "GIF_SPIKE2_ANT",
        Spec(body=minn(maxx(Src0 - (Src0 > Src1), Zero), C0)),
        lambda in0, in1, s0, s1, imm2: np.minimum(
            np.maximum(in0 - (in0 > in1).astype(np.float32), 0.0), s0
        ),
    )
    # v = min(max(v2, th*s0), th*s1)   (v2=Src0, th=Src1, s0=-32, s1=16)
    vclamp = reg(
        "GIF_VCLAMP_ANT",
        Spec(body=minn(maxx(Src0, Src1 * C0), Src1 * C1)),
        lambda in0, in1, s0, s1, imm2: np.minimum(
            np.maximum(in0, in1 * s0), in1 * s1
        ),
    )
    # th' = th*s0 + (s+1)*s1   (th=Src0, s=Src1)
    theta = reg(
        "GIF_THETA_ANT",
        Spec(body=Src0 * C0 + (Src1 + One) * C1),
        lambda in0, in1, s0, s1, imm2: in0 * s0 + (in1 + 1.0) * s1,
    )
    return spike, vclamp, theta


def _build_nc():
    import concourse.bass as bass
    import concourse.mybir as mybir
    from concourse import bacc
    from concourse.masks import make_identity
    from concourse.tile import TileContext

    SPIKE_OP, VCLAMP_OP, THETA_OP = _register_custom_ops()

    f32 = mybir.dt.float32
    Alu = mybir.AluOpType
    Act = mybir.ActivationFunctionType

    nc = bacc.Bacc(trn_type="TRN2")

    x_d = nc.dram_tensor("x", (B_PER, T_LEN, INPUT_DIM), f32, kind="ExternalInput")
    we_d = nc.dram_tensor("W_exc", (EXC_DIM, INPUT_DIM), f32, kind="ExternalInput")
    wi_d = nc.dram_tensor("W_inh", (INH_DIM, INPUT_DIM), f32, kind="ExternalInput")
    br_d = nc.dram_tensor("bias_row", (1, HIDDEN_DIM), f32, kind="ExternalInput")
    sg_d = nc.dram_tensor("sigma", (128, NT), f32, kind="ExternalInput")
    spk_d = nc.dram_tensor(
        "spikes", (B_PER, T_LEN, HIDDEN_DIM), f32, kind="ExternalOutput"
    )
    vf_d = nc.dram_tensor("vf", (B_PER, HIDDEN_DIM), f32, kind="ExternalOutput")
    tf_d = nc.dram_tensor("thf", (B_PER, HIDDEN_DIM), f32, kind="ExternalOutput")

    with TileContext(nc) as tc:
        with (
            tc.tile_pool(name="const", bufs=1) as constp,
            tc.tile_pool(name="wt", bufs=1) as wtp,
            tc.tile_pool(name="state", bufs=1) as statep,
            tc.tile_pool(name="wnat", bufs=2) as wnatp,
            tc.tile_pool(name="xs", bufs=2) as xsp,
            tc.tile_pool(name="xt", bufs=2) as xtp,
            tc.tile_pool(name="io", bufs=2) as iop,
            tc.tile_pool(name="st", bufs=2) as stp,
            tc.tile_pool(name="rl", bufs=2) as rlp,
            tc.tile_pool(name="pw", bufs=2, space="PSUM") as pwp,
            tc.tile_pool(name="pi", bufs=2, space="PSUM") as pip,
        ):
            ident = constp.tile([128, 128], f32, name="ident")
            make_identity(nc, ident)
            sigma = constp.tile([128, NT], f32, name="sigma_sb")
            nc.sync.dma_start(out=sigma, in_=sg_d[:, :])
            brow = constp.tile([1, HIDDEN_DIM], f32, name="brow_sb")
            nc.sync.dma_start(out=brow, in_=br_d[:, :])
            ones = constp.tile([1, B_PER * TC], f32, name="ones_sb")
            nc.vector.memset(ones, 1.0)

            # scan state + scratch, all (128, NT, B_PER)
            v = statep.tile([128, *LANES_F], f32, name="v_st")
            th = statep.tile([128, *LANES_F], f32, name="th_st")
            rth = statep.tile([128, *LANES_F], f32, name="rth_st")
            v1 = statep.tile([128, *LANES_F], f32, name="v1_t")
            q = statep.tile([128, *LANES_F], f32, name="q_t")
            mfr = statep.tile([128, *LANES_F], f32, name="m_t")
            sth = statep.tile([128, *LANES_F], f32, name="sth_t")
            v2 = statep.tile([128, *LANES_F], f32, name="v2_t")
            r1 = statep.tile([128, *LANES_F], f32, name="r1_t")
            nc.vector.memset(v, 0.0)
            nc.vector.memset(th, float(THRESHOLD))
            nc.vector.memset(rth, 1.0 / float(THRESHOLD))

            # ---- W^T staging: WT[kt] is (128 k, 2048 h) ----
            WT = [
                wtp.tile([128, HIDDEN_DIM], f32, name=f"WT{kt}", tag=f"WT{kt}")
                for kt in range(KT)
            ]
            for ht in range(NT):
                wnat = wnatp.tile([128, INPUT_DIM], f32, name="wnat", tag="wnat")
                h0 = ht * 128
                if h0 + 128 <= EXC_DIM:
                    nc.sync.dma_start(out=wnat, in_=we_d[h0 : h0 + 128, :])
                elif h0 >= EXC_DIM:
                    nc.sync.dma_start(
                        out=wnat, in_=wi_d[h0 - EXC_DIM : h0 - EXC_DIM + 128, :]
                    )
                else:
                    ne = EXC_DIM - h0
                    nc.sync.dma_start(out=wnat[:ne, :], in_=we_d[h0:EXC_DIM, :])
                    nc.sync.dma_start(out=wnat[ne:, :], in_=wi_d[: 128 - ne, :])
                for kt in range(KT):
                    pw = pwp.tile([128, 128], f32, name="pw", tag="pw")
                    nc.tensor.transpose(pw, wnat[:, kt * 128 : (kt + 1) * 128], ident)
                    nc.scalar.activation(
                        WT[kt][:, h0 : h0 + 128], pw, Act.Copy
                    )

            # ---- main chunk loop ----
            for c in range(N_CHUNK):
                t0 = c * TC
                xs = []
                for b in range(B_PER):
                    xsb = xsp.tile([128, INPUT_DIM], f32, name=f"xs{b}", tag=f"xs{b}")
                    nc.sync.dma_start(out=xsb, in_=x_d[b, t0 : t0 + TC, :])
                    xs.append(xsb)
                xT = [
                    xtp.tile([128, B_PER * TC], f32, name=f"xT{kt}", tag=f"xT{kt}")
                    for kt in range(KT)
                ]
                for b in range(B_PER):
                    for kt in range(KT):
                        px = pwp.tile([128, 128], f32, name="px", tag="pw")
                        nc.tensor.transpose(
                            px, xs[b][:, kt * 128 : (kt + 1) * 128], ident
                        )
                        nc.scalar.activation(
                            xT[kt][:, b * TC : (b + 1) * TC], px, Act.Copy
                        )

                I = iop.tile([128, NT, B_PER, TC], f32, name="I_sb", tag="I")
                for ht in range(NT):
                    pi = pip.tile([128, B_PER * TC], f32, name="pi", tag="pi")
                    h0 = ht * 128
                    nc.tensor.matmul(
                        pi,
                        brow[:, h0 : h0 + 128],
                        ones,
                        start=True,
                        stop=False,
                    )
                    for kt in range(KT):
                        nc.tensor.matmul(
                            pi,
                            WT[kt][:, h0 : h0 + 128],
                            xT[kt],
                            start=False,
                            stop=(kt == KT - 1),
                        )
                    rl = rlp.tile([128, B_PER * TC], f32, name="rl", tag="rl")
                    nc.scalar.activation(rl, pi, Act.Relu)
                    nc.scalar.activation(
                        I[:, ht, :, :],
                        rl.rearrange("p (b t) -> p b t", b=B_PER),
                        Act.Copy,
                        scale=sigma[:, ht : ht + 1],
                    )

                S = iop.tile([128, NT, B_PER, TC], f32, name="S_sb", tag="S", bufs=1)
                for t in range(TC):
                    it = I[:, :, :, t]
                    st = S[:, :, :, t]
                    nc.vector.scalar_tensor_tensor(
                        v1, v, float(DECAY), it, Alu.mult, Alu.add
                    )
                    nc.vector.tensor_tensor(q, v1, rth, Alu.mult)
                    nc.vector.tensor_scalar(mfr, q, MAGIC, -MAGIC, Alu.add, Alu.add)
                    nc.vector._custom_dve(SPIKE_OP, out=st, in0=mfr, in1=q, s0=L)
                    nc.vector.tensor_tensor(sth, st, th, Alu.mult)
                    nc.vector.tensor_tensor(v2, v1, sth, Alu.subtract)
                    nc.vector._custom_dve(
                        VCLAMP_OP, out=v, in0=v2, in1=th, s0=-2.0 * L, s1=L
                    )
                    nc.vector._custom_dve(
                        THETA_OP, out=th, in0=th, in1=st, s0=1.0 - ALPHA, s1=ALPHA
                    )
                    nc.vector.reciprocal_approx_accurate(out=rth, in_=th, scratch=r1)

                # transpose spikes to (t, h) layout so DRAM descriptors are
                # 8KB-contiguous rows, then DMA out per batch
                ST = stp.tile([128, B_PER, HIDDEN_DIM], f32, name="ST_sb", tag="ST")
                for b in range(B_PER):
                    for ht in range(NT):
                        ps = pwp.tile([128, 128], f32, name="ps", tag="pw")
                        nc.tensor.transpose(ps, S[:, ht, b, :], ident)
                        nc.scalar.activation(
                            ST[:, b, ht * 128 : (ht + 1) * 128], ps, Act.Copy
                        )
                for b in range(B_PER):
                    nc.sync.dma_start(
                        out=spk_d[b, t0 : t0 + TC, :], in_=ST[:, b, :]
                    )

            # final states: transpose (128p, NT) per batch -> (NT, 128) and DMA
            for name, st_tile, dram in (("vf", v, vf_d), ("tf", th, tf_d)):
                for b in range(B_PER):
                    pf = pwp.tile([128, 128], f32, name=f"pf_{name}{b}", tag="pw")
                    nc.tensor.transpose(pf[:NT, :], st_tile[:, :, b], ident)
                    sf = statep.tile([NT, 128], f32, name=f"sf_{name}{b}")
                    nc.scalar.activation(sf, pf[:NT, :], Act.Copy)
                    nc.sync.dma_start(
                        out=dram[b, :].rearrange("(ht p) -> ht p", p=128), in_=sf
                    )

    nc.finalize()
    return nc


def _get_nc():
    if "nc" not in _NC_CACHE:
        _NC_CACHE["nc"] = _build_nc()
    return _NC_CACHE["nc"]


def _install_profile_hook_shim():
    """antenv.axon_hooks is absent in this image; recreate it so trace=True
    can drive NTFF profiling through libaxon_pjrt.so (same logic as
    trn_boot._ntff_profile_via_ctypes)."""
    import sys as _sys
    import types

    if "antenv.axon_hooks" in _sys.modules:
        return
    import importlib.util

    spec = importlib.util.spec_from_file_location(
        "_trn_boot_shim", "/root/.axon_site/trn_agent_boot/trn_boot.py"
    )
    boot = importlib.util.module_from_spec(spec)
    spec.loader.exec_module(boot)
    hook = boot._ntff_profile_via_ctypes("/opt/axon/libaxon_pjrt.so")
    mod = types.ModuleType("antenv.axon_hooks")
    mod._hook = hook
    mod.set_axon_ntff_profile_hook = lambda h: setattr(mod, "_hook", h)
    mod.get_axon_ntff_profile_hook = lambda: mod._hook
    _sys.modules["antenv.axon_hooks"] = mod
    import antenv

    antenv.axon_hooks = mod


def kernel(x, W_exc, b_exc, W_inh, b_inh, trace=False, tmpdir=None):
    from concourse.bass_utils import run_bass_kernel_spmd

    if trace:
        _install_profile_hook_shim()

    x = np.ascontiguousarray(x, dtype=np.float32)
    W_exc = np.ascontiguousarray(W_exc, dtype=np.float32)
    W_inh = np.ascontiguousarray(W_inh, dtype=np.float32)
    bias_row = np.concatenate(
        [np.asarray(b_exc, np.float32), np.asarray(b_inh, np.float32)]
    )[None, :]
    h_idx = np.arange(HIDDEN_DIM).reshape(NT, 128).T  # (128, NT): h = ht*128 + p
    sigma = np.where(h_idx < EXC_DIM, 1.0, -1.0).astype(np.float32)

    nc = _get_nc()
    in_maps = []
    for c in range(N_CORES):
        in_maps.append(
            {
                "x": x[c * B_PER : (c + 1) * B_PER],
                "W_exc": W_exc,
                "W_inh": W_inh,
                "bias_row": bias_row,
                "sigma": sigma,
            }
        )
    res = run_bass_kernel_spmd(
        nc,
        in_maps,
        core_ids=list(range(N_CORES)),
        trace=trace,
        **({"tmpdir": tmpdir} if tmpdir else {}),
    )
    outs = res.results
    spikes = np.concatenate([o["spikes"] for o in outs], axis=0)
    v_f = np.concatenate([o["vf"] for o in outs], axis=0)
    th_f = np.concatenate([o["thf"] for o in outs], axis=0)
    kernel.last_exec_time_ns = res.exec_time_ns
    kernel.last_results = res
    return (spikes, (v_f, th_f))
